# revision 24
# baseline (speedup 1.0000x reference)
"""GraphConv GNN (4-layer + mean-pool + head) on 8 Trainium2 NeuronCores.

v2 strategy:
  - Host relabels nodes into 8 shards x W_CNT windows x 128 slots, balancing
    per-window in-degree (snake deal by degree + refinement).
  - Layers 1-3 aggregate via: AllGather a bf16 node table [npad, 128] to
    DRAM, dma_gather per-edge 256B rows into SBUF (4 SWDGE queues in
    parallel -- desc-gen is the bottleneck engine), scatter-add via bf16
    one-hot matmuls on the PE into PSUM (feature-major agg), fold in the
    root term, bias+ReLU on ScalarE.
  - Layer 4 has no ReLU and feeds only mean-pooling, so its aggregation
    collapses to dense matmuls: pool(A @ h3 @ W4r) = C.T @ (h3 @ W4r)
    where C[s, g] = #edges from node s into graph g (host-built constant).
    Likewise the root term uses the batch one-hot B. No gather needed.
  - Head + bias + mean division on host.
"""

import sys

if "/opt/trn_rl_repo" not in sys.path:
    sys.path.insert(0, "/opt/trn_rl_repo")

import numpy as np


def _ensure_ntff_hook_module():
    """bass_utils imports antenv.axon_hooks for trace=True under axon;
    some containers lack it. Provide a functional stand-in."""
    try:
        import antenv.axon_hooks  # noqa: F401
        return
    except ImportError:
        pass
    import types

    mod = types.ModuleType("antenv.axon_hooks")
    mod._hook = None

    def set_axon_ntff_profile_hook(hook):
        mod._hook = hook

    def get_axon_ntff_profile_hook():
        if mod._hook is None:
            try:
                from trn_agent_boot.trn_boot import _ntff_profile_via_ctypes

                mod._hook = _ntff_profile_via_ctypes(
                    "/opt/axon/libaxon_pjrt.so"
                )
            except Exception:
                return None
        return mod._hook

    mod.set_axon_ntff_profile_hook = set_axon_ntff_profile_hook
    mod.get_axon_ntff_profile_hook = get_axon_ntff_profile_hook
    try:
        import antenv

        antenv.axon_hooks = mod
    except ImportError:
        pass
    sys.modules["antenv.axon_hooks"] = mod


_ensure_ntff_hook_module()

CORES = 8
WIN_P = 128  # nodes per window == SBUF partitions
HID = 64
WG = 2  # windows per gather group
NQ = 4  # SWDGE queues used round-robin for gathers
TBL_W = 128  # table row width in bf16 elements (256B rows)

_PROGRAM_CACHE: dict = {}


# --------------------------------------------------------------------------
# Host-side planning
# --------------------------------------------------------------------------
def _plan(x, src, dst, batch, n_graphs):
    n_nodes = x.shape[0]
    w_cnt = -(-n_nodes // (CORES * WIN_P))  # windows per core
    n_win_tot = CORES * w_cnt
    shard = w_cnt * WIN_P
    npad = CORES * shard
    half = npad // 2
    assert half <= 32768

    indeg = np.bincount(dst, minlength=n_nodes)
    order = np.argsort(-indeg, kind="stable")
    # snake-deal nodes (desc degree) across all windows; slot = deal round
    idxs = np.arange(n_nodes)
    rounds = idxs // n_win_tot
    pos = idxs % n_win_tot
    wsel = np.where(rounds % 2 == 0, pos, n_win_tot - 1 - pos)
    gw = np.empty(n_nodes, np.int64)
    slot = np.empty(n_nodes, np.int64)
    gw[order] = wsel
    slot[order] = rounds
    assert slot.max() < WIN_P

    # refine: 2D greedy vector balancing of (lo,hi) in-degree per window,
    # so the per-(window,half) edge quota can round down to 1024
    for _ in range(2):
        core_of = gw // w_cnt
        src_hi = (core_of[src] >= CORES // 2).astype(np.int64)
        lo_deg = np.zeros(n_nodes, np.int64)
        hi_deg = np.zeros(n_nodes, np.int64)
        np.add.at(lo_deg, dst[src_hi == 0], 1)
        np.add.at(hi_deg, dst[src_hi == 1], 1)
        deg2 = lo_deg + hi_deg
        order2 = np.argsort(-deg2, kind="stable")
        llo = np.zeros(n_win_tot, np.float64)
        lhi = np.zeros(n_win_tot, np.float64)
        ncount = np.zeros(n_win_tot, np.int64)
        gw_new = np.empty(n_nodes, np.int64)
        slot_new = np.empty(n_nodes, np.int64)
        for n in order2:
            score = (llo + lo_deg[n]) ** 2 + (lhi + hi_deg[n]) ** 2
            score[ncount >= WIN_P] = np.inf
            w = int(np.argmin(score))
            gw_new[n] = w
            slot_new[n] = ncount[w]
            ncount[w] += 1
            llo[w] += lo_deg[n]
            lhi[w] += hi_deg[n]
        gw, slot = gw_new, slot_new

    # targeted swap refinement: push every (window,half) cell under 1024
    core_of = gw // w_cnt
    src_hi = (core_of[src] >= CORES // 2).astype(np.int64)
    lo_deg = np.zeros(n_nodes, np.int64)
    hi_deg = np.zeros(n_nodes, np.int64)
    np.add.at(lo_deg, dst[src_hi == 0], 1)
    np.add.at(hi_deg, dst[src_hi == 1], 1)
    llo = np.zeros(n_win_tot, np.int64)
    lhi = np.zeros(n_win_tot, np.int64)
    np.add.at(llo, gw, lo_deg)
    np.add.at(lhi, gw, hi_deg)
    win_nodes = [list(np.where(gw == w)[0]) for w in range(n_win_tot)]
    target = 1024
    for _ in range(4000):
        loads = np.stack([llo, lhi])
        h_star, w_star = np.unravel_index(np.argmax(loads), loads.shape)
        if loads[h_star, w_star] <= target:
            break
        hd = hi_deg if h_star else lo_deg
        od = lo_deg if h_star else hi_deg
        oth = llo if h_star else lhi
        # node in w_star with max h-degree; receiver window minimizing
        # post-swap max of its cells
        cand = win_nodes[w_star]
        a = cand[int(np.argmax(hd[cand]))]
        combined = (loads[h_star] + hd[a]) + 0.25 * (oth + od[a])
        combined[w_star] = np.inf
        # keep node-half labels invariant: only swap within the same
        # half-group of cores
        wins = np.arange(n_win_tot)
        same_side = (wins // w_cnt >= CORES // 2) == (w_star // w_cnt >= CORES // 2)
        combined[~same_side] = np.inf
        w_to = int(np.argmin(combined))
        bcand = win_nodes[w_to]
        b = bcand[int(np.argmin(hd[bcand]))]
        # swap a <-> b
        win_nodes[w_star].remove(a)
        win_nodes[w_to].remove(b)
        win_nodes[w_star].append(b)
        win_nodes[w_to].append(a)
        for h, dgs in ((0, lo_deg), (1, hi_deg)):
            ld = llo if h == 0 else lhi
            ld[w_star] += dgs[b] - dgs[a]
            ld[w_to] += dgs[a] - dgs[b]
        gw[a], gw[b] = w_to, w_star
    # recompute slots from final window membership
    for w in range(n_win_tot):
        for i, n in enumerate(win_nodes[w]):
            slot[n] = i

    core_of = gw // w_cnt
    w_of = gw % w_cnt
    lid = w_of * WIN_P + slot  # local node id within shard
    pid = core_of * shard + lid  # padded global id

    ecore = core_of[dst]
    ew = w_of[dst]
    ehalf = (pid[src] >= half).astype(np.int64)
    edloc = slot[dst]

    cell = (ecore * w_cnt + ew) * 2 + ehalf
    ccounts = np.bincount(cell, minlength=n_win_tot * 2)
    # per-(core,window,half) chunk quota: pad each cell to a 128 multiple
    ccell = ccounts.reshape(CORES, w_cnt, 2)
    qcs = np.maximum(-(-ccell.max(axis=0) // WIN_P), 1).astype(np.int64)

    groups = [list(range(g, min(g + WG, w_cnt))) for g in range(0, w_cnt, WG)]
    s_tot = int(qcs.sum()) * WIN_P

    # slot offset of each (window, half) in the per-core edge array
    off = np.zeros((w_cnt, 2), np.int64)
    o = 0
    for g in groups:
        for h in (0, 1):
            for w in g:
                off[w, h] = o
                o += int(qcs[w, h]) * WIN_P
    assert o == s_tot

    idx16 = np.zeros((CORES, s_tot), np.int16)
    dloc = np.full((CORES, s_tot), -1.0, np.float32)

    eorder = np.argsort(cell, kind="stable")
    sorted_cell = cell[eorder]
    cell_starts = np.zeros(n_win_tot * 2 + 1, np.int64)
    np.cumsum(np.bincount(cell, minlength=n_win_tot * 2), out=cell_starts[1:])
    rank = np.arange(len(eorder)) - cell_starts[sorted_cell]
    p = off[ew[eorder], ehalf[eorder]] + rank
    ec = ecore[eorder]
    idx16[ec, p] = (pid[src] - ehalf * half)[eorder].astype(np.int16)
    dloc[ec, p] = edloc[eorder].astype(np.float32)

    # wrapped index layout: [C, 128, S/16]; 16-partition pattern replicated x8
    idxw = idx16.reshape(CORES, s_tot // 16, 16).transpose(0, 2, 1)
    idx_tile = np.ascontiguousarray(np.tile(idxw, (1, 8, 1)))
    dloc_tile = np.ascontiguousarray(
        dloc.reshape(CORES, s_tot // WIN_P, WIN_P).transpose(0, 2, 1)
    )

    f_in = x.shape[1]
    x_t = np.zeros((CORES, f_in, shard), np.float32)
    x_t[core_of, :, lid] = x.astype(np.float32)
    bpool = np.zeros((CORES, shard, n_graphs), np.float32)
    bpool[core_of, lid, batch] = 1.0

    # C[s, g] = #edges from node s into graph-g destinations (for the
    # layer-4 pooled aggregation C.T @ T4).
    cidx = lid[src] + shard * core_of[src] + npad * batch[dst]
    cfull = np.bincount(cidx, minlength=npad * n_graphs).reshape(n_graphs, npad)
    cmat = np.ascontiguousarray(
        cfull.T.reshape(CORES, shard, n_graphs)
    ).astype(np.float32)

    return dict(
        w_cnt=w_cnt, qcs=tuple(map(tuple, qcs)), shard=shard, npad=npad,
        half=half, groups=groups,
        s_tot=s_tot, idx_tile=idx_tile, dloc_tile=dloc_tile, x_t=x_t,
        bpool=bpool, cmat=cmat, n_graphs=n_graphs,
    )


# --------------------------------------------------------------------------
# Bass program
# --------------------------------------------------------------------------
def _build_program(w_cnt, qcs, n_graphs, f_in=128):
    import concourse.bacc as bacc
    import concourse.mybir as mybir
    from concourse import tile

    dt = mybir.dt
    f32 = dt.float32
    bf16 = dt.bfloat16
    alu = mybir.AluOpType
    act = mybir.ActivationFunctionType

    shard = w_cnt * WIN_P
    npad = CORES * shard
    half = npad // 2
    s_tot = int(sum(qcs[w][h] for w in range(w_cnt) for h in (0, 1))) * WIN_P
    qcmax = max(qcs[w][h] for w in range(w_cnt) for h in (0, 1))
    groups = [list(range(g, min(g + WG, w_cnt))) for g in range(0, w_cnt, WG)]
    gmaxc = max(sum(qcs[w][h] for w in g) for g in groups for h in (0, 1))
    rg = [list(range(CORES))]

    nc = bacc.Bacc(
        "TRN2", target_bir_lowering=False, debug=False,
        enable_asserts=False, num_devices=CORES, num_swdge_queues=NQ,
    )

    def din(name, shape, dtyp=bf16):
        return nc.dram_tensor(name, shape, dtyp, kind="ExternalInput").ap()

    xT = din("xT", [f_in, shard])
    idx = din("idx", [128, s_tot // 16], dt.int16)
    dmt = din("dmat", [128, s_tot // WIN_P, 128])
    bp = din("bpool", [shard, n_graphs])
    cm = din("cmat", [shard, n_graphs])
    ident = din("ident", [128, 128])
    w1r = din("w1_rel", [128, 64])
    w1o = din("w1_root", [128, 64])
    b1 = din("b1", [64, 1], f32)
    w2r = din("w2_rel", [64, 128])
    w2o = din("w2_root", [64, 128])
    b2 = din("b2", [128, 1], f32)
    w3r = din("w3_rel", [128, 192])
    w3o = din("w3_root", [128, 192])
    b3a = din("b3a", [128, 1], f32)
    b3b = din("b3b", [64, 1], f32)
    w4r = din("w4_rel", [192, 64])
    w4o = din("w4_root", [192, 64])
    pooled = nc.dram_tensor(
        "pooled", [n_graphs, HID], f32, kind="ExternalOutput"
    ).ap()

    with tile.TileContext(nc) as tc:
        with (
            tc.tile_pool(name="const", bufs=1) as cp,
            tc.tile_pool(name="hbuf", bufs=3) as hp,
            tc.tile_pool(name="gather", bufs=6) as gp,
            tc.tile_pool(name="dmat", bufs=5) as dp,
            tc.tile_pool(name="stage", bufs=4) as sp,
            tc.tile_pool(name="aggs", bufs=3) as ap_,
            tc.tile_pool(name="dram", bufs=1, space="DRAM") as dram,
            tc.tile_pool(name="ps_agg", bufs=3, space="PSUM") as pagg,
            tc.tile_pool(name="ps_aux", bufs=2, space="PSUM") as paux,
            tc.tile_pool(name="ps_h", bufs=2, space="PSUM") as ph,
            tc.tile_pool(name="ps_pool", bufs=1, space="PSUM") as ppl,
        ):
            # ---- persistent SBUF loads -----------------------------------
            s_idx = cp.tile([128, s_tot // 16], dt.int16)
            nc.sync.dma_start(s_idx[:], idx[:])
            s_id = cp.tile([128, 128], bf16)
            nc.sync.dma_start(s_id[:], ident[:])
            s_B = cp.tile([128, w_cnt, n_graphs], bf16)
            nc.sync.dma_start(
                s_B[:], bp[:].rearrange("(w p) g -> p w g", p=WIN_P))
            s_C = cp.tile([128, w_cnt, n_graphs], bf16)
            nc.sync.dma_start(
                s_C[:], cm[:].rearrange("(w p) g -> p w g", p=WIN_P))

            def load(apx, shape, dtyp=bf16):
                t = cp.tile(shape, dtyp, name=f"w_{apx.tensor.name}")
                nc.sync.dma_start(t[:], apx[:])
                return t

            s_w1r = load(w1r, [128, 64])
            s_w1o = load(w1o, [128, 64])
            s_b1 = load(b1, [64, 1], f32)
            s_w2r = load(w2r, [64, 128])
            s_w2o = load(w2o, [64, 128])
            s_b2 = load(b2, [128, 1], f32)
            s_w3r = load(w3r, [128, 192])
            s_w3o = load(w3o, [128, 192])
            s_b3a = load(b3a, [128, 1], f32)
            s_b3b = load(b3b, [64, 1], f32)
            # w4 needs K=192 split across partitions
            s_w4ra = cp.tile([128, 64], bf16)
            nc.sync.dma_start(s_w4ra[:], w4r[0:128, :])
            s_w4rb = cp.tile([64, 64], bf16)
            nc.sync.dma_start(s_w4rb[:], w4r[128:192, :])
            s_w4oa = cp.tile([128, 64], bf16)
            nc.sync.dma_start(s_w4oa[:], w4o[0:128, :])
            s_w4ob = cp.tile([64, 64], bf16)
            nc.sync.dma_start(s_w4ob[:], w4o[128:192, :])

            s_xT = hp.tile([f_in, shard], bf16, tag="hbuf")
            nc.sync.dma_start(s_xT[:], xT[:])

            # ---- DRAM bounce + shared tables (bf16, 256B rows) -----------
            tbl_in = []
            tbl = []
            for i in range(3):
                ti = dram.tile([shard, TBL_W], bf16, name=f"tblin{i}")
                tf = dram.tile([npad, TBL_W], bf16, name=f"tbl{i}",
                               addr_space="Shared")
                tbl_in.append(ti)
                tbl.append(tf)

            qctr = [0]  # SWDGE queue rotation

            # ---- helper: one aggregation pass ----------------------------
            def agg_pass(layer, width, mk_stages, pre_mm=None):
                """mk_stages(w, ps) -> list of stage closures; stage s of
                window j is issued after window j+s+1's scatter matmuls
                (software pipelining so PE ops never wait on fresh
                Scalar/Vector results). pre_mm(w, ps) opens the PSUM
                accumulation group (start=True) before the scatters."""
                table = tbl[layer]
                t_lo = table[0:half, :]
                t_hi = table[half:npad, :]
                win_stages = []
                ran = []

                def pump(emitted):
                    for j in range(len(win_stages)):
                        while (ran[j] < len(win_stages[j])
                               and j + ran[j] + 1 <= emitted):
                            win_stages[j][ran[j]]()
                            ran[j] += 1

                off = 0
                widx = 0
                for g in groups:
                    nw = len(g)
                    num = nw * q
                    cols = num // WIN_P
                    v_lo = gp.tile([128, gmaxc, TBL_W], bf16, tag="glo",
                                   name=f"glo{layer}_{g[0]}")
                    v_hi = gp.tile([128, gmaxc, TBL_W], bf16, tag="ghi",
                                   name=f"ghi{layer}_{g[0]}")
                    # one-hot slab for the whole group (lo+hi cells)
                    dm = dp.tile([128, 2 * gmaxc, 128], bf16, tag="dmat",
                                 name=f"d{layer}_{g[0]}")
                    nc.scalar.dma_start(
                        dm[:, 0:2 * nw * qc, :],
                        dmt[:, off // WIN_P: (off + 2 * num) // WIN_P, :])
                    off2 = off + num
                    nc.gpsimd.dma_gather(
                        v_lo[:, 0:cols, :],
                        t_lo, s_idx[:, off // 16: off2 // 16],
                        num, num, TBL_W, elem_step=TBL_W,
                        single_packet=False, queue_num=qctr[0] % NQ,
                    )
                    qctr[0] += 1
                    nc.gpsimd.dma_gather(
                        v_hi[:, 0:cols, :],
                        t_hi, s_idx[:, off2 // 16: (off2 + num) // 16],
                        num, num, TBL_W, elem_step=TBL_W,
                        single_packet=False, queue_num=qctr[0] % NQ,
                    )
                    qctr[0] += 1
                    for wi, w in enumerate(g):
                        ps = pagg.tile([width, 128], f32, tag="agg",
                                       name=f"agg{layer}_{w}")
                        n_mm = 2 * qc
                        mm = 0
                        if pre_mm is not None:
                            pre_mm(w, ps)
                        for hh, gbuf in enumerate((v_lo, v_hi)):
                            cell = hh * nw + wi
                            for k in range(qc):
                                is_last = mm == n_mm - 1
                                nc.tensor.matmul(
                                    ps[:],
                                    gbuf[:, wi * qc + k, 0:width],
                                    dm[:, cell * qc + k, :],
                                    start=(mm == 0 and pre_mm is None),
                                    stop=is_last,
                                    skip_group_check=True,
                                )
                                mm += 1
                        win_stages.append(mk_stages(w, ps))
                        ran.append(0)
                        widx += 1
                        pump(widx - 1)
                    off = off2 + num
                pump(10 ** 9)

            # ---- L1 table prepass: T1 = x @ w1_rel (node-major) ----------
            for w in range(w_cnt):
                ws = slice(w * WIN_P, (w + 1) * WIN_P)
                ps_p = paux.tile([128, 64], f32, tag="aux", bufs=1, name=f"p1_{w}")
                nc.tensor.matmul(ps_p[:], s_xT[:, ws], s_w1r[:],
                                 start=True, stop=True)
                st = sp.tile([128, 64], bf16, tag="stage", name=f"t1s_{w}")
                nc.vector.tensor_copy(st[:], ps_p[:])
                nc.sync.dma_start(tbl_in[0][ws, 0:64], st[:])
            nc.gpsimd.collective_compute(
                "AllGather", alu.bypass, replica_groups=rg,
                ins=[tbl_in[0].opt()], outs=[tbl[0].opt()],
            )

            h1T = hp.tile([64, shard], bf16, tag="hbuf")

            def pre1(w, ps):
                ws = slice(w * WIN_P, (w + 1) * WIN_P)
                nc.tensor.matmul(ps[:], s_w1o[:], s_xT[:, ws],
                                 start=True, stop=False, skip_group_check=True)

            def stages1(w, ps):
                ws = slice(w * WIN_P, (w + 1) * WIN_P)

                def s0():
                    nc.scalar.activation(h1T[:, ws], ps[:], act.Relu,
                                         bias=s_b1[:])

                def s1():
                    # T2 = h1 node-major via PE transpose
                    ps_t = paux.tile([128, 64], bf16, tag="auxt", bufs=1,
                                     name=f"t2p_{w}")
                    nc.tensor.transpose(ps_t[:], h1T[:, ws], s_id[0:64, 0:64])
                    st = sp.tile([128, 64], bf16, tag="stage", name=f"t2s_{w}")
                    nc.vector.tensor_copy(st[:], ps_t[:])
                    nc.sync.dma_start(tbl_in[1][ws, 0:64], st[:])

                return [s0, s1]

            agg_pass(0, 64, stages1, pre_mm=pre1)
            nc.gpsimd.collective_compute(
                "AllGather", alu.bypass, replica_groups=rg,
                ins=[tbl_in[1].opt()], outs=[tbl[1].opt()],
            )

            h2T = hp.tile([128, shard], bf16, tag="hbuf")

            def stages2(w, ps):
                ws = slice(w * WIN_P, (w + 1) * WIN_P)
                ag = ap_.tile([64, 128], bf16, tag="aggs", name=f"ag2_{w}")

                def s0():
                    nc.scalar.activation(ag[:], ps[:], act.Identity)

                def s1():
                    ps_h = ph.tile([128, 128], f32, tag="psh", bufs=1,
                                   name=f"h2p_{w}")
                    nc.tensor.matmul(ps_h[:], s_w2r[:], ag[:],
                                     start=True, stop=False)
                    nc.tensor.matmul(ps_h[:], s_w2o[:], h1T[:, ws],
                                     start=False, stop=True)
                    nc.scalar.activation(h2T[:, ws], ps_h[:], act.Relu,
                                         bias=s_b2[:])

                def s2():
                    # T3 = h2 node-major (bf16)
                    ps_t = paux.tile([128, 128], bf16, tag="auxt", bufs=1,
                                     name=f"t3p_{w}")
                    nc.tensor.transpose(ps_t[:], h2T[:, ws], s_id[:])
                    st = sp.tile([128, 128], bf16, tag="stageb",
                                 name=f"t3s_{w}")
                    nc.vector.tensor_copy(st[:], ps_t[:])
                    nc.sync.dma_start(tbl_in[2][ws, :], st[:])

                return [s0, s1, s2]

            agg_pass(1, 64, stages2)
            nc.gpsimd.collective_compute(
                "AllGather", alu.bypass, replica_groups=rg,
                ins=[tbl_in[2].opt()], outs=[tbl[2].opt()],
            )

            ps_g = ppl.tile([n_graphs, HID], f32)

            def stages3(w, ps):
                ws = slice(w * WIN_P, (w + 1) * WIN_P)
                ag = ap_.tile([128, 128], bf16, tag="aggs3", name=f"ag3_{w}")
                h3a = sp.tile([128, 128], bf16, tag="h3a", name=f"h3a_{w}")
                h3b = sp.tile([64, 128], bf16, tag="h3b", name=f"h3b_{w}")
                t4s = sp.tile([128, 64], bf16, tag="t4s", name=f"t4s_{w}")
                us = sp.tile([128, 64], bf16, tag="us", name=f"us_{w}")

                def s0():
                    nc.scalar.activation(ag[:], ps[:], act.Identity)

                def s1():
                    ps_a = ph.tile([128, 128], f32, tag="psh", bufs=1,
                                   name=f"h3ap_{w}")
                    nc.tensor.matmul(ps_a[:], s_w3r[:, 0:128], ag[:],
                                     start=True, stop=False)
                    nc.tensor.matmul(ps_a[:], s_w3o[:, 0:128], h2T[:, ws],
                                     start=False, stop=True)
                    nc.scalar.activation(h3a[:], ps_a[:], act.Relu,
                                         bias=s_b3a[:])
                    ps_b = paux.tile([64, 128], f32, tag="aux", bufs=1,
                                     name=f"h3bp_{w}")
                    nc.tensor.matmul(ps_b[:], s_w3r[:, 128:192], ag[:],
                                     start=True, stop=False)
                    nc.tensor.matmul(ps_b[:], s_w3o[:, 128:192], h2T[:, ws],
                                     start=False, stop=True)
                    nc.scalar.activation(h3b[:], ps_b[:], act.Relu,
                                         bias=s_b3b[:])

                def s2():
                    # layer 4, fused: T4 = h3 @ w4_rel, U = h3 @ w4_root
                    ps_t4 = ph.tile([128, 64], f32, tag="pst4", bufs=1,
                                    name=f"t4_{w}")
                    nc.tensor.matmul(ps_t4[:], h3a[:], s_w4ra[:],
                                     start=True, stop=False)
                    nc.tensor.matmul(ps_t4[:], h3b[:], s_w4rb[:],
                                     start=False, stop=True)
                    nc.scalar.activation(t4s[:], ps_t4[:], act.Identity)
                    ps_u = ph.tile([128, 64], f32, tag="pst4", bufs=1,
                                   name=f"u_{w}")
                    nc.tensor.matmul(ps_u[:], h3a[:], s_w4oa[:],
                                     start=True, stop=False)
                    nc.tensor.matmul(ps_u[:], h3b[:], s_w4ob[:],
                                     start=False, stop=True)
                    nc.scalar.activation(us[:], ps_u[:], act.Identity)

                def s3():
                    nc.tensor.matmul(ps_g[:], s_C[:, w, :], t4s[:],
                                     start=(w == 0), stop=False,
                                     skip_group_check=True)
                    nc.tensor.matmul(ps_g[:], s_B[:, w, :], us[:],
                                     start=False, stop=(w == w_cnt - 1),
                                     skip_group_check=True)

                return [s0, s1, s2, s3]

            agg_pass(2, 128, stages3)

            s_out = sp.tile([n_graphs, HID], f32, tag="out")
            nc.vector.tensor_copy(s_out[:], ps_g[:])
            nc.sync.dma_start(pooled[:], s_out[:])

    nc.compile()
    return nc


def _get_program(w_cnt, qcs, n_graphs, f_in):
    key = (w_cnt, qcs, n_graphs, f_in)
    if key not in _PROGRAM_CACHE:
        _PROGRAM_CACHE[key] = _build_program(w_cnt, qcs, n_graphs, f_in)
    return _PROGRAM_CACHE[key]


# --------------------------------------------------------------------------
# Execution
# --------------------------------------------------------------------------
def _in_maps(plan, inputs):
    import ml_dtypes

    bf = ml_dtypes.bfloat16
    maps = []
    ident = np.eye(128, dtype=np.float32)
    w3 = np.asarray(inputs["b3"], np.float32)
    def b16(a):
        return np.ascontiguousarray(np.asarray(a, np.float32)).astype(bf)

    iota3 = b16(np.tile(np.arange(128, dtype=np.float32), (128, 24)))

    for c in range(CORES):
        m = {
            "xT": b16(plan["x_t"][c]),
            "idx": plan["idx_tile"][c],
            "dloc": b16(plan["dloc_tile"][c]),
            "iota3": iota3,
            "bpool": b16(plan["bpool"][c]),
            "cmat": b16(plan["cmat"][c]),
            "ident": b16(ident),
            "w1_rel": b16(inputs["w1_rel"]),
            "w1_root": b16(inputs["w1_root"]),
            "b1": np.asarray(inputs["b1"], np.float32).reshape(-1, 1),
            "w2_rel": b16(inputs["w2_rel"]),
            "w2_root": b16(inputs["w2_root"]),
            "b2": np.asarray(inputs["b2"], np.float32).reshape(-1, 1),
            "w3_rel": b16(inputs["w3_rel"]),
            "w3_root": b16(inputs["w3_root"]),
            "b3a": w3[:128].reshape(-1, 1),
            "b3b": w3[128:].reshape(-1, 1),
            "w4_rel": b16(inputs["w4_rel"]),
            "w4_root": b16(inputs["w4_root"]),
        }
        maps.append(m)
    return maps


def _post(outs, inputs, n_graphs):
    total = np.zeros((n_graphs, HID), np.float32)
    for o in outs:
        total += np.asarray(o["pooled"], np.float32)
    batch = np.asarray(inputs["batch"]).astype(np.int64)
    counts = np.bincount(batch, minlength=n_graphs).astype(np.float32)
    b4 = np.asarray(inputs["b4"], np.float32)
    total += counts[:, None] * b4[None, :]
    pooled = total / np.maximum(counts, 1.0)[:, None]
    hw = np.asarray(inputs["head_w"], np.float32)
    hb = np.asarray(inputs["head_b"], np.float32)
    return (pooled @ hw + hb).astype(np.float32)


def run(inputs, trace=False, sim=False, n_graphs=64):
    x = np.asarray(inputs["x"], np.float32)
    ei = np.asarray(inputs["edge_index"]).astype(np.int64)
    batch = np.asarray(inputs["batch"]).astype(np.int64)
    plan = _plan(x, ei[0], ei[1], batch, n_graphs)
    nc = _get_program(plan["w_cnt"], plan["qcs"], n_graphs, x.shape[1])
    maps = _in_maps(plan, inputs)

    if sim:
        from concourse.bass_interp import MultiCoreSim

        msim = MultiCoreSim(nc, num_cores=CORES)
        for c in range(CORES):
            for k, v in maps[c].items():
                msim.cores[c].tensor(k)[:] = v
        msim.simulate()
        outs = [
            {"pooled": np.array(msim.cores[c].tensor("pooled"))}
            for c in range(CORES)
        ]
        return _post(outs, inputs, n_graphs), None

    from concourse import bass_utils

    res = bass_utils.run_bass_kernel_spmd(
        nc, maps, core_ids=list(range(CORES)), trace=trace,
    )
    out = _post(res.results, inputs, n_graphs)
    return out, res


def kernel(**inputs) -> np.ndarray:
    out, _ = run(inputs)
    return out


# revision 25
# speedup vs baseline: 1.0071x; 1.0071x over previous
"""GraphConv GNN (4-layer + mean-pool + head) on 8 Trainium2 NeuronCores.

v2 strategy:
  - Host relabels nodes into 8 shards x W_CNT windows x 128 slots, balancing
    per-window in-degree (snake deal by degree + refinement).
  - Layers 1-3 aggregate via: AllGather a bf16 node table [npad, 128] to
    DRAM, dma_gather per-edge 256B rows into SBUF (4 SWDGE queues in
    parallel -- desc-gen is the bottleneck engine), scatter-add via bf16
    one-hot matmuls on the PE into PSUM (feature-major agg), fold in the
    root term, bias+ReLU on ScalarE.
  - Layer 4 has no ReLU and feeds only mean-pooling, so its aggregation
    collapses to dense matmuls: pool(A @ h3 @ W4r) = C.T @ (h3 @ W4r)
    where C[s, g] = #edges from node s into graph g (host-built constant).
    Likewise the root term uses the batch one-hot B. No gather needed.
  - Head + bias + mean division on host.
"""

import sys

if "/opt/trn_rl_repo" not in sys.path:
    sys.path.insert(0, "/opt/trn_rl_repo")

import numpy as np


def _ensure_ntff_hook_module():
    """bass_utils imports antenv.axon_hooks for trace=True under axon;
    some containers lack it. Provide a functional stand-in."""
    try:
        import antenv.axon_hooks  # noqa: F401
        return
    except ImportError:
        pass
    import types

    mod = types.ModuleType("antenv.axon_hooks")
    mod._hook = None

    def set_axon_ntff_profile_hook(hook):
        mod._hook = hook

    def get_axon_ntff_profile_hook():
        if mod._hook is None:
            try:
                from trn_agent_boot.trn_boot import _ntff_profile_via_ctypes

                mod._hook = _ntff_profile_via_ctypes(
                    "/opt/axon/libaxon_pjrt.so"
                )
            except Exception:
                return None
        return mod._hook

    mod.set_axon_ntff_profile_hook = set_axon_ntff_profile_hook
    mod.get_axon_ntff_profile_hook = get_axon_ntff_profile_hook
    try:
        import antenv

        antenv.axon_hooks = mod
    except ImportError:
        pass
    sys.modules["antenv.axon_hooks"] = mod


_ensure_ntff_hook_module()

CORES = 8
WIN_P = 128  # nodes per window == SBUF partitions
HID = 64
WG = 2  # windows per gather group
NQ = 4  # SWDGE queues used round-robin for gathers
TBL_W = 128  # table row width in bf16 elements (256B rows)

_PROGRAM_CACHE: dict = {}


# --------------------------------------------------------------------------
# Host-side planning
# --------------------------------------------------------------------------
def _plan(x, src, dst, batch, n_graphs):
    n_nodes = x.shape[0]
    w_cnt = -(-n_nodes // (CORES * WIN_P))  # windows per core
    n_win_tot = CORES * w_cnt
    shard = w_cnt * WIN_P
    npad = CORES * shard
    half = npad // 2
    assert half <= 32768

    indeg = np.bincount(dst, minlength=n_nodes)
    order = np.argsort(-indeg, kind="stable")
    # snake-deal nodes (desc degree) across all windows; slot = deal round
    idxs = np.arange(n_nodes)
    rounds = idxs // n_win_tot
    pos = idxs % n_win_tot
    wsel = np.where(rounds % 2 == 0, pos, n_win_tot - 1 - pos)
    gw = np.empty(n_nodes, np.int64)
    slot = np.empty(n_nodes, np.int64)
    gw[order] = wsel
    slot[order] = rounds
    assert slot.max() < WIN_P

    # refine: 2D greedy vector balancing of (lo,hi) in-degree per window,
    # so the per-(window,half) edge quota can round down to 1024
    for _ in range(2):
        core_of = gw // w_cnt
        src_hi = (core_of[src] >= CORES // 2).astype(np.int64)
        lo_deg = np.zeros(n_nodes, np.int64)
        hi_deg = np.zeros(n_nodes, np.int64)
        np.add.at(lo_deg, dst[src_hi == 0], 1)
        np.add.at(hi_deg, dst[src_hi == 1], 1)
        deg2 = lo_deg + hi_deg
        order2 = np.argsort(-deg2, kind="stable")
        llo = np.zeros(n_win_tot, np.float64)
        lhi = np.zeros(n_win_tot, np.float64)
        ncount = np.zeros(n_win_tot, np.int64)
        gw_new = np.empty(n_nodes, np.int64)
        slot_new = np.empty(n_nodes, np.int64)
        for n in order2:
            score = (llo + lo_deg[n]) ** 2 + (lhi + hi_deg[n]) ** 2
            score[ncount >= WIN_P] = np.inf
            w = int(np.argmin(score))
            gw_new[n] = w
            slot_new[n] = ncount[w]
            ncount[w] += 1
            llo[w] += lo_deg[n]
            lhi[w] += hi_deg[n]
        gw, slot = gw_new, slot_new

    # targeted swap refinement: push every (window,half) cell under 1024
    core_of = gw // w_cnt
    src_hi = (core_of[src] >= CORES // 2).astype(np.int64)
    lo_deg = np.zeros(n_nodes, np.int64)
    hi_deg = np.zeros(n_nodes, np.int64)
    np.add.at(lo_deg, dst[src_hi == 0], 1)
    np.add.at(hi_deg, dst[src_hi == 1], 1)
    llo = np.zeros(n_win_tot, np.int64)
    lhi = np.zeros(n_win_tot, np.int64)
    np.add.at(llo, gw, lo_deg)
    np.add.at(lhi, gw, hi_deg)
    win_nodes = [list(np.where(gw == w)[0]) for w in range(n_win_tot)]
    target = 1024
    for _ in range(4000):
        loads = np.stack([llo, lhi])
        h_star, w_star = np.unravel_index(np.argmax(loads), loads.shape)
        if loads[h_star, w_star] <= target:
            break
        hd = hi_deg if h_star else lo_deg
        od = lo_deg if h_star else hi_deg
        oth = llo if h_star else lhi
        # node in w_star with max h-degree; receiver window minimizing
        # post-swap max of its cells
        cand = win_nodes[w_star]
        a = cand[int(np.argmax(hd[cand]))]
        combined = (loads[h_star] + hd[a]) + 0.25 * (oth + od[a])
        combined[w_star] = np.inf
        # keep node-half labels invariant: only swap within the same
        # half-group of cores
        wins = np.arange(n_win_tot)
        same_side = (wins // w_cnt >= CORES // 2) == (w_star // w_cnt >= CORES // 2)
        combined[~same_side] = np.inf
        w_to = int(np.argmin(combined))
        bcand = win_nodes[w_to]
        b = bcand[int(np.argmin(hd[bcand]))]
        # swap a <-> b
        win_nodes[w_star].remove(a)
        win_nodes[w_to].remove(b)
        win_nodes[w_star].append(b)
        win_nodes[w_to].append(a)
        for h, dgs in ((0, lo_deg), (1, hi_deg)):
            ld = llo if h == 0 else lhi
            ld[w_star] += dgs[b] - dgs[a]
            ld[w_to] += dgs[a] - dgs[b]
        gw[a], gw[b] = w_to, w_star
    # recompute slots from final window membership
    for w in range(n_win_tot):
        for i, n in enumerate(win_nodes[w]):
            slot[n] = i

    core_of = gw // w_cnt
    w_of = gw % w_cnt
    lid = w_of * WIN_P + slot  # local node id within shard
    pid = core_of * shard + lid  # padded global id

    ecore = core_of[dst]
    ew = w_of[dst]
    ehalf = (pid[src] >= half).astype(np.int64)
    edloc = slot[dst]

    cell = (ecore * w_cnt + ew) * 2 + ehalf
    ccounts = np.bincount(cell, minlength=n_win_tot * 2)
    # per-(core,window,half) chunk quota: pad each cell to a 128 multiple
    ccell = ccounts.reshape(CORES, w_cnt, 2)
    qcs = np.maximum(-(-ccell.max(axis=0) // WIN_P), 1).astype(np.int64)

    groups = [list(range(g, min(g + WG, w_cnt))) for g in range(0, w_cnt, WG)]
    s_tot = int(qcs.sum()) * WIN_P

    # slot offset of each (window, half) in the per-core edge array
    off = np.zeros((w_cnt, 2), np.int64)
    o = 0
    for g in groups:
        for h in (0, 1):
            for w in g:
                off[w, h] = o
                o += int(qcs[w, h]) * WIN_P
    assert o == s_tot

    idx16 = np.zeros((CORES, s_tot), np.int16)
    dloc = np.full((CORES, s_tot), -1.0, np.float32)

    eorder = np.argsort(cell, kind="stable")
    sorted_cell = cell[eorder]
    cell_starts = np.zeros(n_win_tot * 2 + 1, np.int64)
    np.cumsum(np.bincount(cell, minlength=n_win_tot * 2), out=cell_starts[1:])
    rank = np.arange(len(eorder)) - cell_starts[sorted_cell]
    p = off[ew[eorder], ehalf[eorder]] + rank
    ec = ecore[eorder]
    idx16[ec, p] = (pid[src] - ehalf * half)[eorder].astype(np.int16)
    dloc[ec, p] = edloc[eorder].astype(np.float32)

    # wrapped index layout: [C, 128, S/16]; 16-partition pattern replicated x8
    idxw = idx16.reshape(CORES, s_tot // 16, 16).transpose(0, 2, 1)
    idx_tile = np.ascontiguousarray(np.tile(idxw, (1, 8, 1)))
    dloc_tile = np.ascontiguousarray(
        dloc.reshape(CORES, s_tot // WIN_P, WIN_P).transpose(0, 2, 1)
    )

    f_in = x.shape[1]
    x_t = np.zeros((CORES, f_in, shard), np.float32)
    x_t[core_of, :, lid] = x.astype(np.float32)
    bpool = np.zeros((CORES, shard, n_graphs), np.float32)
    bpool[core_of, lid, batch] = 1.0

    # C[s, g] = #edges from node s into graph-g destinations (for the
    # layer-4 pooled aggregation C.T @ T4).
    cidx = lid[src] + shard * core_of[src] + npad * batch[dst]
    cfull = np.bincount(cidx, minlength=npad * n_graphs).reshape(n_graphs, npad)
    cmat = np.ascontiguousarray(
        cfull.T.reshape(CORES, shard, n_graphs)
    ).astype(np.float32)

    return dict(
        w_cnt=w_cnt, qcs=tuple(map(tuple, qcs)), shard=shard, npad=npad,
        half=half, groups=groups,
        s_tot=s_tot, idx_tile=idx_tile, dloc_tile=dloc_tile, x_t=x_t,
        bpool=bpool, cmat=cmat, n_graphs=n_graphs,
    )


# --------------------------------------------------------------------------
# Bass program
# --------------------------------------------------------------------------
def _build_program(w_cnt, qcs, n_graphs, f_in=128):
    import concourse.bacc as bacc
    import concourse.mybir as mybir
    from concourse import tile

    dt = mybir.dt
    f32 = dt.float32
    bf16 = dt.bfloat16
    alu = mybir.AluOpType
    act = mybir.ActivationFunctionType

    shard = w_cnt * WIN_P
    npad = CORES * shard
    half = npad // 2
    s_tot = int(sum(qcs[w][h] for w in range(w_cnt) for h in (0, 1))) * WIN_P
    qcmax = max(qcs[w][h] for w in range(w_cnt) for h in (0, 1))
    groups = [list(range(g, min(g + WG, w_cnt))) for g in range(0, w_cnt, WG)]
    gmaxc = max(sum(qcs[w][h] for w in g) for g in groups for h in (0, 1))
    rg = [list(range(CORES))]

    nc = bacc.Bacc(
        "TRN2", target_bir_lowering=False, debug=False,
        enable_asserts=False, num_devices=CORES, num_swdge_queues=NQ,
    )

    def din(name, shape, dtyp=bf16):
        return nc.dram_tensor(name, shape, dtyp, kind="ExternalInput").ap()

    xT = din("xT", [f_in, shard])
    idx = din("idx", [128, s_tot // 16], dt.int16)
    dmt = din("dmat", [128, s_tot // WIN_P, 128])
    bp = din("bpool", [shard, n_graphs])
    cm = din("cmat", [shard, n_graphs])
    ident = din("ident", [128, 128])
    w1r = din("w1_rel", [128, 64])
    w1o = din("w1_root", [128, 64])
    b1 = din("b1", [64, 1], f32)
    w2r = din("w2_rel", [64, 128])
    w2o = din("w2_root", [64, 128])
    b2 = din("b2", [128, 1], f32)
    w3r = din("w3_rel", [128, 192])
    w3o = din("w3_root", [128, 192])
    b3a = din("b3a", [128, 1], f32)
    b3b = din("b3b", [64, 1], f32)
    w4r = din("w4_rel", [192, 64])
    w4o = din("w4_root", [192, 64])
    pooled = nc.dram_tensor(
        "pooled", [n_graphs, HID], f32, kind="ExternalOutput"
    ).ap()

    with tile.TileContext(nc) as tc:
        with (
            tc.tile_pool(name="const", bufs=1) as cp,
            tc.tile_pool(name="hbuf", bufs=3) as hp,
            tc.tile_pool(name="gather", bufs=6) as gp,
            tc.tile_pool(name="dmat", bufs=5) as dp,
            tc.tile_pool(name="stage", bufs=3) as sp,
            tc.tile_pool(name="aggs", bufs=2) as ap_,
            tc.tile_pool(name="dram", bufs=1, space="DRAM") as dram,
            tc.tile_pool(name="ps_agg", bufs=2, space="PSUM") as pagg,
            tc.tile_pool(name="ps_aux", bufs=2, space="PSUM") as paux,
            tc.tile_pool(name="ps_h", bufs=2, space="PSUM") as ph,
            tc.tile_pool(name="ps_pool", bufs=1, space="PSUM") as ppl,
        ):
            # ---- persistent SBUF loads -----------------------------------
            s_idx = cp.tile([128, s_tot // 16], dt.int16)
            nc.sync.dma_start(s_idx[:], idx[:])
            s_id = cp.tile([128, 128], bf16)
            nc.sync.dma_start(s_id[:], ident[:])
            s_B = cp.tile([128, w_cnt, n_graphs], bf16)
            nc.sync.dma_start(
                s_B[:], bp[:].rearrange("(w p) g -> p w g", p=WIN_P))
            s_C = cp.tile([128, w_cnt, n_graphs], bf16)
            nc.sync.dma_start(
                s_C[:], cm[:].rearrange("(w p) g -> p w g", p=WIN_P))

            def load(apx, shape, dtyp=bf16):
                t = cp.tile(shape, dtyp, name=f"w_{apx.tensor.name}")
                nc.sync.dma_start(t[:], apx[:])
                return t

            s_w1r = load(w1r, [128, 64])
            s_w1o = load(w1o, [128, 64])
            s_b1 = load(b1, [64, 1], f32)
            s_w2r = load(w2r, [64, 128])
            s_w2o = load(w2o, [64, 128])
            s_b2 = load(b2, [128, 1], f32)
            s_w3r = load(w3r, [128, 192])
            s_w3o = load(w3o, [128, 192])
            s_b3a = load(b3a, [128, 1], f32)
            s_b3b = load(b3b, [64, 1], f32)
            # w4 needs K=192 split across partitions
            s_w4ra = cp.tile([128, 64], bf16)
            nc.sync.dma_start(s_w4ra[:], w4r[0:128, :])
            s_w4rb = cp.tile([64, 64], bf16)
            nc.sync.dma_start(s_w4rb[:], w4r[128:192, :])
            s_w4oa = cp.tile([128, 64], bf16)
            nc.sync.dma_start(s_w4oa[:], w4o[0:128, :])
            s_w4ob = cp.tile([64, 64], bf16)
            nc.sync.dma_start(s_w4ob[:], w4o[128:192, :])

            s_xT = hp.tile([f_in, shard], bf16, tag="hbuf")
            nc.sync.dma_start(s_xT[:], xT[:])

            # ---- DRAM bounce + shared tables (bf16, 256B rows) -----------
            tbl_in = []
            tbl = []
            for i in range(3):
                ti = dram.tile([shard, TBL_W], bf16, name=f"tblin{i}")
                tf = dram.tile([npad, TBL_W], bf16, name=f"tbl{i}",
                               addr_space="Shared")
                tbl_in.append(ti)
                tbl.append(tf)

            qctr = [0]  # SWDGE queue rotation

            # ---- helper: one aggregation pass ----------------------------
            def agg_pass(layer, width, mk_stages, pre_mm=None):
                """mk_stages(w, ps) -> list of stage closures; stage s of
                window j is issued after window j+s+1's scatter matmuls
                (software pipelining so PE ops never wait on fresh
                Scalar/Vector results). pre_mm(w, ps) opens the PSUM
                accumulation group (start=True) before the scatters."""
                table = tbl[layer]
                t_lo = table[0:half, :]
                t_hi = table[half:npad, :]
                win_stages = []
                ran = []

                def pump(emitted):
                    for j in range(len(win_stages)):
                        while (ran[j] < len(win_stages[j])
                               and j + ran[j] + 1 <= emitted):
                            win_stages[j][ran[j]]()
                            ran[j] += 1

                off = 0
                widx = 0
                for g in groups:
                    nw = len(g)
                    num = nw * q
                    cols = num // WIN_P
                    v_lo = gp.tile([128, gmaxc, TBL_W], bf16, tag="glo",
                                   name=f"glo{layer}_{g[0]}")
                    v_hi = gp.tile([128, gmaxc, TBL_W], bf16, tag="ghi",
                                   name=f"ghi{layer}_{g[0]}")
                    # one-hot slab for the whole group (lo+hi cells)
                    dm = dp.tile([128, 2 * gmaxc, 128], bf16, tag="dmat",
                                 name=f"d{layer}_{g[0]}")
                    nc.scalar.dma_start(
                        dm[:, 0:2 * nw * qc, :],
                        dmt[:, off // WIN_P: (off + 2 * num) // WIN_P, :])
                    off2 = off + num
                    nc.gpsimd.dma_gather(
                        v_lo[:, 0:cols, :],
                        t_lo, s_idx[:, off // 16: off2 // 16],
                        num, num, TBL_W, elem_step=TBL_W,
                        single_packet=False, queue_num=qctr[0] % NQ,
                    )
                    qctr[0] += 1
                    nc.gpsimd.dma_gather(
                        v_hi[:, 0:cols, :],
                        t_hi, s_idx[:, off2 // 16: (off2 + num) // 16],
                        num, num, TBL_W, elem_step=TBL_W,
                        single_packet=False, queue_num=qctr[0] % NQ,
                    )
                    qctr[0] += 1
                    for wi, w in enumerate(g):
                        ps = pagg.tile([width, 128], f32, tag="agg",
                                       name=f"agg{layer}_{w}")
                        n_mm = 2 * qc
                        mm = 0
                        if pre_mm is not None:
                            pre_mm(w, ps)
                        for hh, gbuf in enumerate((v_lo, v_hi)):
                            cell = hh * nw + wi
                            for k in range(qc):
                                is_last = mm == n_mm - 1
                                nc.tensor.matmul(
                                    ps[:],
                                    gbuf[:, wi * qc + k, 0:width],
                                    dm[:, cell * qc + k, :],
                                    start=(mm == 0 and pre_mm is None),
                                    stop=is_last,
                                    skip_group_check=True,
                                )
                                mm += 1
                        win_stages.append(mk_stages(w, ps))
                        ran.append(0)
                        widx += 1
                        pump(widx - 1)
                    off = off2 + num
                pump(10 ** 9)

            # ---- L1 table prepass: T1 = x @ w1_rel (node-major) ----------
            for w in range(w_cnt):
                ws = slice(w * WIN_P, (w + 1) * WIN_P)
                ps_p = paux.tile([128, 64], f32, tag="aux", bufs=1, name=f"p1_{w}")
                nc.tensor.matmul(ps_p[:], s_xT[:, ws], s_w1r[:],
                                 start=True, stop=True)
                st = sp.tile([128, 64], bf16, tag="stage", name=f"t1s_{w}")
                nc.vector.tensor_copy(st[:], ps_p[:])
                nc.sync.dma_start(tbl_in[0][ws, 0:64], st[:])
            nc.gpsimd.collective_compute(
                "AllGather", alu.bypass, replica_groups=rg,
                ins=[tbl_in[0].opt()], outs=[tbl[0].opt()],
            )

            h1T = hp.tile([64, shard], bf16, tag="hbuf")

            def pre1(w, ps):
                ws = slice(w * WIN_P, (w + 1) * WIN_P)
                nc.tensor.matmul(ps[:], s_w1o[:], s_xT[:, ws],
                                 start=True, stop=False, skip_group_check=True)

            def stages1(w, ps):
                ws = slice(w * WIN_P, (w + 1) * WIN_P)

                def s0():
                    nc.scalar.activation(h1T[:, ws], ps[:], act.Relu,
                                         bias=s_b1[:])

                def s1():
                    # T2 = h1 node-major via PE transpose
                    ps_t = paux.tile([128, 64], bf16, tag="auxt", bufs=1,
                                     name=f"t2p_{w}")
                    nc.tensor.transpose(ps_t[:], h1T[:, ws], s_id[0:64, 0:64])
                    st = sp.tile([128, 64], bf16, tag="stage", name=f"t2s_{w}")
                    nc.vector.tensor_copy(st[:], ps_t[:])
                    nc.sync.dma_start(tbl_in[1][ws, 0:64], st[:])

                return [s0, s1]

            agg_pass(0, 64, stages1, pre_mm=pre1)
            nc.gpsimd.collective_compute(
                "AllGather", alu.bypass, replica_groups=rg,
                ins=[tbl_in[1].opt()], outs=[tbl[1].opt()],
            )

            h2T = hp.tile([128, shard], bf16, tag="hbuf")

            def stages2(w, ps):
                ws = slice(w * WIN_P, (w + 1) * WIN_P)
                ag = ap_.tile([64, 128], bf16, tag="aggs", name=f"ag2_{w}")

                def s0():
                    nc.scalar.activation(ag[:], ps[:], act.Identity)

                def s1():
                    ps_h = ph.tile([128, 128], f32, tag="psh", bufs=1,
                                   name=f"h2p_{w}")
                    nc.tensor.matmul(ps_h[:], s_w2r[:], ag[:],
                                     start=True, stop=False)
                    nc.tensor.matmul(ps_h[:], s_w2o[:], h1T[:, ws],
                                     start=False, stop=True)
                    nc.scalar.activation(h2T[:, ws], ps_h[:], act.Relu,
                                         bias=s_b2[:])

                def s2():
                    # T3 = h2 node-major (bf16)
                    ps_t = paux.tile([128, 128], bf16, tag="auxt", bufs=1,
                                     name=f"t3p_{w}")
                    nc.tensor.transpose(ps_t[:], h2T[:, ws], s_id[:])
                    st = sp.tile([128, 128], bf16, tag="stageb",
                                 name=f"t3s_{w}")
                    nc.vector.tensor_copy(st[:], ps_t[:])
                    nc.sync.dma_start(tbl_in[2][ws, :], st[:])

                return [s0, s1, s2]

            agg_pass(1, 64, stages2)
            nc.gpsimd.collective_compute(
                "AllGather", alu.bypass, replica_groups=rg,
                ins=[tbl_in[2].opt()], outs=[tbl[2].opt()],
            )

            ps_g = ppl.tile([n_graphs, HID], f32)

            def stages3(w, ps):
                ws = slice(w * WIN_P, (w + 1) * WIN_P)
                ag = ap_.tile([128, 128], bf16, tag="aggs3", name=f"ag3_{w}")
                h3a = sp.tile([128, 128], bf16, tag="h3a", name=f"h3a_{w}")
                h3b = sp.tile([64, 128], bf16, tag="h3b", name=f"h3b_{w}")
                t4s = sp.tile([128, 64], bf16, tag="t4s", name=f"t4s_{w}")
                us = sp.tile([128, 64], bf16, tag="us", name=f"us_{w}")

                def s0():
                    nc.scalar.activation(ag[:], ps[:], act.Identity)

                def s1():
                    ps_a = ph.tile([128, 128], f32, tag="psh", bufs=1,
                                   name=f"h3ap_{w}")
                    nc.tensor.matmul(ps_a[:], s_w3r[:, 0:128], ag[:],
                                     start=True, stop=False)
                    nc.tensor.matmul(ps_a[:], s_w3o[:, 0:128], h2T[:, ws],
                                     start=False, stop=True)
                    nc.scalar.activation(h3a[:], ps_a[:], act.Relu,
                                         bias=s_b3a[:])
                    ps_b = paux.tile([64, 128], f32, tag="aux", bufs=1,
                                     name=f"h3bp_{w}")
                    nc.tensor.matmul(ps_b[:], s_w3r[:, 128:192], ag[:],
                                     start=True, stop=False)
                    nc.tensor.matmul(ps_b[:], s_w3o[:, 128:192], h2T[:, ws],
                                     start=False, stop=True)
                    nc.scalar.activation(h3b[:], ps_b[:], act.Relu,
                                         bias=s_b3b[:])

                def s2():
                    # layer 4, fused: T4 = h3 @ w4_rel, U = h3 @ w4_root
                    ps_t4 = ph.tile([128, 64], f32, tag="pst4", bufs=1,
                                    name=f"t4_{w}")
                    nc.tensor.matmul(ps_t4[:], h3a[:], s_w4ra[:],
                                     start=True, stop=False)
                    nc.tensor.matmul(ps_t4[:], h3b[:], s_w4rb[:],
                                     start=False, stop=True)
                    nc.scalar.activation(t4s[:], ps_t4[:], act.Identity)
                    ps_u = ph.tile([128, 64], f32, tag="psu", bufs=1,
                                   name=f"u_{w}")
                    nc.tensor.matmul(ps_u[:], h3a[:], s_w4oa[:],
                                     start=True, stop=False)
                    nc.tensor.matmul(ps_u[:], h3b[:], s_w4ob[:],
                                     start=False, stop=True)
                    nc.scalar.activation(us[:], ps_u[:], act.Identity)

                def s3():
                    nc.tensor.matmul(ps_g[:], s_C[:, w, :], t4s[:],
                                     start=(w == 0), stop=False,
                                     skip_group_check=True)
                    nc.tensor.matmul(ps_g[:], s_B[:, w, :], us[:],
                                     start=False, stop=(w == w_cnt - 1),
                                     skip_group_check=True)

                return [s0, s1, s2, s3]

            agg_pass(2, 128, stages3)

            s_out = sp.tile([n_graphs, HID], f32, tag="out")
            nc.vector.tensor_copy(s_out[:], ps_g[:])
            nc.sync.dma_start(pooled[:], s_out[:])

    nc.compile()
    return nc


def _get_program(w_cnt, qcs, n_graphs, f_in):
    key = (w_cnt, qcs, n_graphs, f_in)
    if key not in _PROGRAM_CACHE:
        _PROGRAM_CACHE[key] = _build_program(w_cnt, qcs, n_graphs, f_in)
    return _PROGRAM_CACHE[key]


# --------------------------------------------------------------------------
# Execution
# --------------------------------------------------------------------------
def _in_maps(plan, inputs):
    import ml_dtypes

    bf = ml_dtypes.bfloat16
    maps = []
    ident = np.eye(128, dtype=np.float32)
    w3 = np.asarray(inputs["b3"], np.float32)
    def b16(a):
        return np.ascontiguousarray(np.asarray(a, np.float32)).astype(bf)

    iota3 = b16(np.tile(np.arange(128, dtype=np.float32), (128, 24)))

    for c in range(CORES):
        m = {
            "xT": b16(plan["x_t"][c]),
            "idx": plan["idx_tile"][c],
            "dloc": b16(plan["dloc_tile"][c]),
            "iota3": iota3,
            "bpool": b16(plan["bpool"][c]),
            "cmat": b16(plan["cmat"][c]),
            "ident": b16(ident),
            "w1_rel": b16(inputs["w1_rel"]),
            "w1_root": b16(inputs["w1_root"]),
            "b1": np.asarray(inputs["b1"], np.float32).reshape(-1, 1),
            "w2_rel": b16(inputs["w2_rel"]),
            "w2_root": b16(inputs["w2_root"]),
            "b2": np.asarray(inputs["b2"], np.float32).reshape(-1, 1),
            "w3_rel": b16(inputs["w3_rel"]),
            "w3_root": b16(inputs["w3_root"]),
            "b3a": w3[:128].reshape(-1, 1),
            "b3b": w3[128:].reshape(-1, 1),
            "w4_rel": b16(inputs["w4_rel"]),
            "w4_root": b16(inputs["w4_root"]),
        }
        maps.append(m)
    return maps


def _post(outs, inputs, n_graphs):
    total = np.zeros((n_graphs, HID), np.float32)
    for o in outs:
        total += np.asarray(o["pooled"], np.float32)
    batch = np.asarray(inputs["batch"]).astype(np.int64)
    counts = np.bincount(batch, minlength=n_graphs).astype(np.float32)
    b4 = np.asarray(inputs["b4"], np.float32)
    total += counts[:, None] * b4[None, :]
    pooled = total / np.maximum(counts, 1.0)[:, None]
    hw = np.asarray(inputs["head_w"], np.float32)
    hb = np.asarray(inputs["head_b"], np.float32)
    return (pooled @ hw + hb).astype(np.float32)


def run(inputs, trace=False, sim=False, n_graphs=64):
    x = np.asarray(inputs["x"], np.float32)
    ei = np.asarray(inputs["edge_index"]).astype(np.int64)
    batch = np.asarray(inputs["batch"]).astype(np.int64)
    plan = _plan(x, ei[0], ei[1], batch, n_graphs)
    nc = _get_program(plan["w_cnt"], plan["qcs"], n_graphs, x.shape[1])
    maps = _in_maps(plan, inputs)

    if sim:
        from concourse.bass_interp import MultiCoreSim

        msim = MultiCoreSim(nc, num_cores=CORES)
        for c in range(CORES):
            for k, v in maps[c].items():
                msim.cores[c].tensor(k)[:] = v
        msim.simulate()
        outs = [
            {"pooled": np.array(msim.cores[c].tensor("pooled"))}
            for c in range(CORES)
        ]
        return _post(outs, inputs, n_graphs), None

    from concourse import bass_utils

    res = bass_utils.run_bass_kernel_spmd(
        nc, maps, core_ids=list(range(CORES)), trace=trace,
    )
    out = _post(res.results, inputs, n_graphs)
    return out, res


def kernel(**inputs) -> np.ndarray:
    out, _ = run(inputs)
    return out
